# revision 4
# baseline (speedup 1.0000x reference)
"""Trainium2 Bass kernel for Gemma3 sliding-window attention.

Problem: B=1, T=4096, d_model=2048, 8 query heads / 4 KV heads, head_dim=256,
sliding window 1024, per-head RMSNorm + RoPE (interleaved rotate-half with
cat(freqs,freqs) tables), o_proj.

Sharding (8 cores): 4 KV-head groups x 2 sequence halves. Core (g, s) computes
query heads {2g, 2g+1} and KV head g for query tokens [s*2048, (s+1)*2048),
with a 1024-token KV halo (recomputed locally; s=0's halo is zero-padded and
masked out via the exp bias). Each core emits a partial o-projection
[2048, 2048] in bf16; the host sums the 4 group partials per half in f32.

v1 changes vs baseline:
- Host pre-arranges x/weights/tables into the exact SBUF tile layouts so every
  DMA is a contiguous 2D transfer.
- Three DMA queues: sync (SP-HWDGE) carries the latency-critical stream
  (wk/wv, x tiles, rope tables, small consts) in deadline order; gpsimd
  (SW-DGE) carries bulky latency-tolerant weights (wq, tri, wo) and all
  output writes, so outputs never delay input prefetches.
- K projection runs ko-inner so the first matmuls start after only half of
  wk plus half of the first x tile have landed.
- o-projection partials are written as bf16 (half the output traffic).
- RMSNorm's square runs on the vector engine (one less ACT table, less ACT).
"""

import sys

if "/opt/trn_rl_repo" not in sys.path:
    sys.path.insert(0, "/opt/trn_rl_repo")

import numpy as np

try:
    import ml_dtypes
    BF16 = ml_dtypes.bfloat16
except ImportError:
    BF16 = None

T, DM, NH, NKV, HD, WIN = 4096, 2048, 8, 4, 256, 1024
EPS, BASE = 1e-6, 10000.0
NG, NS = 4, 2
TL, NQ = 3072, 2048
NTB = 6           # 512-token K/V tiles per core
NKO = 16          # 2048 / 128 contraction subtiles
SCALE = 1.0 / 16.0
NEG = -1.0e5

_cache = {}


def _host_prep(x, pos, Wq, Wk, Wv, Wo, q_norm_w, k_norm_w):
    x = np.asarray(x, np.float32).reshape(T, DM)
    xT = np.ascontiguousarray(x.T)
    pos_f = np.asarray(pos).astype(np.float64)
    m = np.arange(128)
    invf = BASE ** (-m / 128.0)

    Wq = np.asarray(Wq, np.float32)
    Wk = np.asarray(Wk, np.float32)
    Wv = np.asarray(Wv, np.float32)
    Wo = np.asarray(Wo, np.float32)
    qnw = np.asarray(q_norm_w, np.float32)
    knw = np.asarray(k_norm_w, np.float32)

    ones = np.ones((128, 128), np.float32)
    r0T = np.zeros((128, 128), np.float32)
    a = np.arange(64)
    r0T[2 * a, 2 * a + 1] = 1.0
    r0T[2 * a + 1, 2 * a] = -1.0
    qw2 = np.ascontiguousarray(np.stack([qnw[:128], qnw[128:]], axis=1))
    kw2 = np.ascontiguousarray(np.stack([knw[:128], knw[128:]], axis=1))

    # masks for 512-wide attention blocks: m=0..3 far edge, m=8..11 diagonal
    jp = np.arange(128)[:, None]
    ip = np.arange(512)[None, :]
    tris = []
    for mm_ in range(4):
        tris.append(jp >= ip + 1 - 128 * mm_)         # far masks F_m
    for mm_ in range(4):
        tris.append(jp <= ip - 128 * mm_)             # diag masks D_{m+8}
    tri = np.concatenate(tris, axis=1).astype(BF16)   # [128, 8*512]

    in_maps = []
    for g in range(NG):
        wkT = Wk[g * HD:(g + 1) * HD, :].T            # [DM, 256]
        wvT = Wv[g * HD:(g + 1) * HD, :].T
        wqT = Wq[2 * g * HD:(2 * g + 2) * HD, :].T    # [DM, 512]
        woT = Wo[:, 2 * g * HD:(2 * g + 2) * HD].T    # [512, DM]
        wkp = np.ascontiguousarray(
            wkT.reshape(NKO, 128, 256).transpose(1, 0, 2)).astype(BF16)
        wvp = np.ascontiguousarray(
            wvT.reshape(NKO, 128, 256).transpose(1, 0, 2)).astype(BF16)
        wqp = np.ascontiguousarray(
            wqT.reshape(NKO, 128, 512).transpose(1, 0, 2)).astype(BF16)
        wop = np.ascontiguousarray(
            woT.reshape(4, 128, DM).transpose(1, 0, 2)).astype(BF16)
        for s in range(NS):
            lo = s * 2048 - 1024
            xT_c = np.zeros((DM, TL), np.float32)
            src_lo = max(lo, 0)
            xT_c[:, src_lo - lo:] = xT[:, src_lo:(s + 1) * 2048]
            # [12, 128, 8, 512]: tile (tb, half) -> ko = half*8..half*8+7
            xp = np.ascontiguousarray(
                xT_c.reshape(NKO, 128, NTB, 512)
                .transpose(2, 0, 1, 3)                 # [tb, ko, p, t]
                .reshape(NTB, 2, 8, 128, 512)
                .transpose(0, 1, 3, 2, 4)              # [tb, half, p, k, t]
                .reshape(NTB * 2, 128, 8, 512)).astype(BF16)

            pidx = np.clip(np.arange(lo, lo + TL), 0, T - 1)
            p = pos_f[pidx]
            p[np.arange(lo, lo + TL) < 0] = 0.0
            ang = p[None, :] * invf[:, None]
            cosk = np.cos(ang).astype(np.float32)      # [128, TL]
            sink = np.sin(ang).astype(np.float32)
            cosp = np.ascontiguousarray(
                cosk.reshape(128, NTB, 512).transpose(1, 0, 2))
            sinp = np.ascontiguousarray(
                sink.reshape(128, NTB, 512).transpose(1, 0, 2))

            kbias = np.zeros((128, 24), np.float32)
            if s == 0:
                kbias[:, :8] = NEG

            in_maps.append({
                "xp": xp,
                "cosp": cosp,
                "sinp": sinp,
                "wkp": wkp,
                "wvp": wvp,
                "wqp": wqp,
                "wop": wop,
                "ones_bf": ones.astype(BF16),
                "r0T": r0T.astype(BF16),
                "qw": qw2,
                "kw": kw2,
                "kbias": kbias,
                "tri": tri,
            })
    return in_maps


def _build_program():
    if "nc" in _cache:
        return _cache["nc"]

    import concourse.bass as bass
    import concourse.mybir as mybir
    import concourse.tile as tile
    from concourse import bacc
    from contextlib import ExitStack

    f32 = mybir.dt.float32
    bf16 = mybir.dt.bfloat16
    AF = mybir.ActivationFunctionType
    OP = mybir.AluOpType

    nc = bacc.Bacc("TRN2", target_bir_lowering=False, debug=False,
                   enable_asserts=False, num_devices=8)

    xp_d = nc.dram_tensor("xp", [NTB * 2, 128, 8, 512], bf16, kind="ExternalInput")
    cosp_d = nc.dram_tensor("cosp", [NTB, 128, 512], f32, kind="ExternalInput")
    sinp_d = nc.dram_tensor("sinp", [NTB, 128, 512], f32, kind="ExternalInput")
    wq_d = nc.dram_tensor("wqp", [128, NKO, 512], bf16, kind="ExternalInput")
    wk_d = nc.dram_tensor("wkp", [128, NKO, 256], bf16, kind="ExternalInput")
    wv_d = nc.dram_tensor("wvp", [128, NKO, 256], bf16, kind="ExternalInput")
    wo_d = nc.dram_tensor("wop", [128, 4, DM], bf16, kind="ExternalInput")
    onesbf_d = nc.dram_tensor("ones_bf", [128, 128], bf16, kind="ExternalInput")
    r0_d = nc.dram_tensor("r0T", [128, 128], bf16, kind="ExternalInput")
    qw_d = nc.dram_tensor("qw", [128, 2], f32, kind="ExternalInput")
    kw_d = nc.dram_tensor("kw", [128, 2], f32, kind="ExternalInput")
    kb_d = nc.dram_tensor("kbias", [128, 24], f32, kind="ExternalInput")
    tri_d = nc.dram_tensor("tri", [128, 8 * 512], bf16, kind="ExternalInput")
    o_d = nc.dram_tensor("o_part", [NQ, DM], bf16, kind="ExternalOutput")

    with tile.TileContext(nc) as tc, ExitStack() as ctx:
        cpool = ctx.enter_context(tc.tile_pool(name="consts", bufs=1))
        xpool = ctx.enter_context(tc.tile_pool(name="xt", bufs=5))
        tabpool = ctx.enter_context(tc.tile_pool(name="tab", bufs=4))
        kpool = ctx.enter_context(tc.tile_pool(name="kring", bufs=5))
        vpool = ctx.enter_context(tc.tile_pool(name="vring", bufs=5))
        scpool = ctx.enter_context(tc.tile_pool(name="scratch", bufs=3))
        spool = ctx.enter_context(tc.tile_pool(name="small", bufs=2))
        qpool = ctx.enter_context(tc.tile_pool(name="qt", bufs=3))
        ptpool = ctx.enter_context(tc.tile_pool(name="pt", bufs=4))
        ypool = ctx.enter_context(tc.tile_pool(name="yt", bufs=3))
        opool = ctx.enter_context(tc.tile_pool(name="osb", bufs=3))
        pp_proj = ctx.enter_context(tc.tile_pool(name="pproj", bufs=3, space="PSUM"))
        pp_small = ctx.enter_context(tc.tile_pool(name="psmall", bufs=2, space="PSUM"))
        pp_acc = ctx.enter_context(tc.tile_pool(name="pacc", bufs=3, space="PSUM"))

        # ---- sync queue: deadline-ordered latency-critical stream ----
        wk_sb = cpool.tile([128, NKO, 256], bf16, tag="wk")
        nc.sync.dma_start(wk_sb[:, 0:8, :], wk_d.ap()[:, 0:8, :])
        xt_tiles = [None] * (NTB * 2)
        for i in range(2):
            xt = xpool.tile([128, 8, 512], bf16, tag="xt", name=f"xt_p{i}")
            nc.sync.dma_start(xt[:], xp_d.ap()[i])
            xt_tiles[i] = xt
        nc.sync.dma_start(wk_sb[:, 8:16, :], wk_d.ap()[:, 8:16, :])
        wv_sb = cpool.tile([128, NKO, 256], bf16, tag="wv")
        nc.sync.dma_start(wv_sb[:, 0:8, :], wv_d.ap()[:, 0:8, :])
        nc.sync.dma_start(wv_sb[:, 8:16, :], wv_d.ap()[:, 8:16, :])
        ones_sb = cpool.tile([128, 128], bf16, tag="ones")
        nc.sync.dma_start(ones_sb[:], onesbf_d.ap())
        r0_sb = cpool.tile([128, 128], bf16, tag="r0")
        nc.sync.dma_start(r0_sb[:], r0_d.ap())
        qw_sb = cpool.tile([128, 2], f32, tag="qwt")
        nc.sync.dma_start(qw_sb[:], qw_d.ap())
        kw_sb = cpool.tile([128, 2], f32, tag="kwt")
        nc.sync.dma_start(kw_sb[:], kw_d.ap())
        kb_sb = cpool.tile([128, 24], f32, tag="kb")
        nc.sync.dma_start(kb_sb[:], kb_d.ap())
        cos_tiles = [None] * NTB
        sin_tiles = [None] * NTB

        def load_tab(tb):
            ct = tabpool.tile([128, 512], f32, tag="cos", name=f"cos{tb}")
            nc.sync.dma_start(ct[:], cosp_d.ap()[tb])
            st = tabpool.tile([128, 512], f32, tag="sin", name=f"sin{tb}")
            nc.sync.dma_start(st[:], sinp_d.ap()[tb])
            cos_tiles[tb] = ct
            sin_tiles[tb] = st

        load_tab(0)
        for i in range(2, 4):
            xt = xpool.tile([128, 8, 512], bf16, tag="xt", name=f"xt_p{i}")
            nc.sync.dma_start(xt[:], xp_d.ap()[i])
            xt_tiles[i] = xt
        load_tab(1)

        # ---- gpsimd queue: bulky latency-tolerant weights ----
        wq_sb = cpool.tile([128, NKO, 512], bf16, tag="wq")
        nc.gpsimd.dma_start(wq_sb[:, 0:8, :], wq_d.ap()[:, 0:8, :])
        nc.gpsimd.dma_start(wq_sb[:, 8:16, :], wq_d.ap()[:, 8:16, :])
        tri_sb = cpool.tile([128, 8 * 512], bf16, tag="tri")
        nc.gpsimd.dma_start(tri_sb[:], tri_d.ap())
        wo_sb = cpool.tile([128, 4, DM], bf16, tag="wo")
        nc.gpsimd.dma_start(wo_sb[:, 0:2, :], wo_d.ap()[:, 0:2, :])
        nc.gpsimd.dma_start(wo_sb[:, 2:4, :], wo_d.ap()[:, 2:4, :])

        eps_sb = cpool.tile([128, 1], f32, tag="eps")
        nc.vector.memset(eps_sb[:], EPS)
        zero_sb = cpool.tile([128, 1], f32, tag="zero")
        nc.vector.memset(zero_sb[:], 0.0)

        kt_tiles = [None] * NTB
        vt_tiles = [None] * NTB

        def norm_rope(src_ps, w_sb, cos_t, sin_t, dst, dsti):
            """src_ps: two PSUM [128, 512] tiles (one head's 2 d-subtiles),
            transposed projection over 512 tokens. Writes RMSNorm+RoPE (bf16)
            into dst[:, dsti+u, :]."""
            z2 = scpool.tile([128, 2, 512], bf16, tag="z2")
            for u in range(2):
                nc.scalar.activation(z2[:, u, :], src_ps[u][:], AF.Square,
                                     bias=zero_sb[:])
            ssq = pp_small.tile([128, 512], f32, tag="psm")
            for u in range(2):
                nc.tensor.matmul(ssq[:], ones_sb[:], z2[:, u, :],
                                 start=(u == 0), stop=(u == 1))
            sq = spool.tile([128, 512], f32, tag="sq")
            nc.scalar.activation(sq[:], ssq[:], AF.Sqrt, bias=eps_sb[:], scale=1.0 / HD)
            rs = spool.tile([128, 512], f32, tag="rs")
            nc.vector.reciprocal_approx_fast(rs[:], sq[:])
            znw = scpool.tile([128, 2, 512], bf16, tag="znw")
            t1 = scpool.tile([128, 2, 512], f32, tag="t1")
            for u in range(2):
                nc.vector.scalar_tensor_tensor(
                    znw[:, u, :], src_ps[u][:], w_sb[:, u:u + 1], rs[:],
                    OP.mult, OP.mult)
                rot = pp_small.tile([128, 512], f32, tag="psm")
                nc.tensor.matmul(rot[:], r0_sb[:], znw[:, u, :], start=True, stop=True)
                nc.vector.tensor_tensor(t1[:, u, :], znw[:, u, :], cos_t, OP.mult)
                tmp = spool.tile([128, 512], f32, tag="tmp")
                nc.vector.tensor_tensor(tmp[:], rot[:], sin_t, OP.mult)
                nc.vector.tensor_tensor(dst[:, dsti + u, :], t1[:, u, :], tmp[:], OP.add)

        for tb in range(NTB):
            # prefetch: x tiles 2 tbs ahead, rope tables 2 tbs ahead
            pf = tb + 2
            if pf < NTB:
                for half in range(2):
                    i = pf * 2 + half
                    xt = xpool.tile([128, 8, 512], bf16, tag="xt", name=f"xt_p{i}")
                    nc.sync.dma_start(xt[:], xp_d.ap()[i])
                    xt_tiles[i] = xt
                load_tab(pf)

            xth = xt_tiles[tb * 2:tb * 2 + 2]
            cos_t = cos_tiles[tb]
            sin_t = sin_tiles[tb]

            # ---- k projection (transposed, N=512), ko-inner ----
            k0_ps = pp_proj.tile([128, 512], f32, tag="pj")
            k1_ps = pp_proj.tile([128, 512], f32, tag="pj")
            k_ps = [k0_ps, k1_ps]
            for ko in range(NKO):
                for dsub in range(2):
                    nc.tensor.matmul(k_ps[dsub][:],
                                     wk_sb[:, ko, dsub * 128:(dsub + 1) * 128],
                                     xth[ko // 8][:, ko % 8, :],
                                     start=(ko == 0), stop=(ko == NKO - 1),
                                     skip_group_check=True)
            kt = kpool.tile([128, 2, 512], bf16, tag="kt")
            norm_rope(k_ps, kw_sb, cos_t[:], sin_t[:], kt, 0)
            kt_tiles[tb] = kt

            # ---- v projection (natural layout) ----
            vt = vpool.tile([128, 4, 256], bf16, tag="vt")
            for vh in range(2):
                v_ps = pp_proj.tile([128, 2, 256], f32, tag="pj")
                for ms in range(2):
                    msub = vh * 2 + ms
                    for ko in range(NKO):
                        nc.tensor.matmul(v_ps[:, ms, :],
                                         xth[ko // 8][:, ko % 8, msub * 128:(msub + 1) * 128],
                                         wv_sb[:, ko, :],
                                         start=(ko == 0), stop=(ko == NKO - 1))
                for ms in range(2):
                    nc.vector.tensor_copy(vt[:, vh * 2 + ms, :], v_ps[:, ms, :])
            vt_tiles[tb] = vt

            if tb < 2:
                continue

            # ---- q projections (2 heads, N=512) ----
            qt_sb = qpool.tile([128, 4, 512], bf16, tag="q")
            for h in range(2):
                q0_ps = pp_proj.tile([128, 512], f32, tag="pj")
                q1_ps = pp_proj.tile([128, 512], f32, tag="pj")
                q_ps = [q0_ps, q1_ps]
                for u in range(2):
                    dsub = 2 * h + u
                    for ko in range(NKO):
                        nc.tensor.matmul(q_ps[u][:],
                                         wq_sb[:, ko, dsub * 128:(dsub + 1) * 128],
                                         xth[ko // 8][:, ko % 8, :],
                                         start=(ko == 0), stop=(ko == NKO - 1))
                norm_rope(q_ps, qw_sb, cos_t[:], sin_t[:], qt_sb, 2 * h)

            # ---- attention for 512-query block a ----
            a = tb - 2
            yt_sb = ypool.tile([128, 4, 512], bf16, tag="y")
            for h in range(2):
                dn_ps = pp_acc.tile([128, 512], f32, tag="pac")
                y0_ps = pp_acc.tile([128, 512], f32, tag="pac")
                y1_ps = pp_acc.tile([128, 512], f32, tag="pac")
                y_ps = [y0_ps, y1_ps]
                for mi, mrel in enumerate([3, 0, 1, 2] + list(range(4, 12))):
                    jt = 4 * a + mrel
                    ct, jh = jt // 4, jt % 4
                    ktc = kt_tiles[ct]
                    vtc = vt_tiles[ct]
                    # active query range: edge tiles are mostly masked
                    if mrel <= 2:
                        ia, ib = 0, 128 * (mrel + 1)
                    elif mrel >= 9:
                        ia, ib = 128 * (mrel - 8), 512
                    else:
                        ia, ib = 0, 512
                    pt = ptpool.tile([128, 512], bf16, tag="p")
                    st = pp_small.tile([128, 512], f32, tag="psm")
                    for u in range(2):
                        nc.tensor.matmul(st[:, ia:ib],
                                         ktc[:, u, jh * 128:(jh + 1) * 128],
                                         qt_sb[:, 2 * h + u, ia:ib],
                                         start=(u == 0), stop=(u == 1))
                    nc.scalar.activation(pt[:, ia:ib], st[:, ia:ib], AF.Exp,
                                         bias=kb_sb[:, jt:jt + 1], scale=SCALE)
                    if mrel < 4:
                        nc.vector.tensor_tensor(
                            pt[:, ia:ib], pt[:, ia:ib],
                            tri_sb[:, mrel * 512 + ia:mrel * 512 + ib], OP.mult)
                    elif mrel >= 8:
                        nc.vector.tensor_tensor(
                            pt[:, ia:ib], pt[:, ia:ib],
                            tri_sb[:, (mrel - 4) * 512 + ia:(mrel - 4) * 512 + ib],
                            OP.mult)
                    first, last = (mi == 0), (mrel == 11)
                    nc.tensor.matmul(dn_ps[:, ia:ib], ones_sb[:], pt[:, ia:ib],
                                     start=first, stop=last, skip_group_check=True)
                    for dh in range(2):
                        nc.tensor.matmul(y_ps[dh][:, ia:ib],
                                         vtc[:, jh, dh * 128:(dh + 1) * 128],
                                         pt[:, ia:ib], start=first, stop=last,
                                         skip_group_check=True)
                rc = spool.tile([128, 512], f32, tag="rc")
                nc.vector.reciprocal_approx_fast(rc[:], dn_ps[:])
                for dh in range(2):
                    nc.vector.tensor_tensor(yt_sb[:, 2 * h + dh, :],
                                            y_ps[dh][:], rc[:], OP.mult)

            # ---- partial o-projection for the 512-query block ----
            for msub in range(4):
                for dmh in range(2):
                    o_sb = opool.tile([128, 1024], bf16, tag="o")
                    for dq in range(2):
                        c0 = (dmh * 2 + dq) * 512
                        o_ps = pp_small.tile([128, 512], f32, tag="psm")
                        for hd in range(4):
                            nc.tensor.matmul(o_ps[:],
                                             yt_sb[:, hd, msub * 128:(msub + 1) * 128],
                                             wo_sb[:, hd, c0:c0 + 512],
                                             start=(hd == 0), stop=(hd == 3))
                        nc.scalar.copy(o_sb[:, dq * 512:(dq + 1) * 512], o_ps[:])
                    r0_ = a * 512 + msub * 128
                    nc.gpsimd.dma_start(o_d.ap()[r0_:r0_ + 128, dmh * 1024:(dmh + 1) * 1024],
                                        o_sb[:])

    nc.compile()
    _cache["nc"] = nc
    return nc


def _run(inputs, trace=False):
    from concourse.bass_utils import run_bass_kernel_spmd

    nc = _build_program()
    in_maps = _host_prep(**inputs)
    res = run_bass_kernel_spmd(nc, in_maps, core_ids=list(range(8)), trace=trace)
    full = np.zeros((T, DM), np.float32)
    for g in range(NG):
        for s in range(NS):
            full[s * 2048:(s + 1) * 2048] += res.results[g * 2 + s]["o_part"].astype(np.float32)
    return full.reshape(1, T, DM), res


def kernel(**inputs):
    return _run(inputs, trace=False)[0]


# revision 9
# speedup vs baseline: 1.0696x; 1.0696x over previous
"""Trainium2 Bass kernel for Gemma3 sliding-window attention. v2: halo exchange.

Sharding (8 cores): 4 KV-head groups x 2 sequence halves. Core (g, s) computes
query heads {2g, 2g+1} and KV head g for query tokens [s*2048, (s+1)*2048).
Unlike v1 (which recomputed a 1024-token KV halo locally from zero-padded x),
each core projects K/V only for its OWN 2048 tokens; the halo K/V for the
second-half cores is delivered by a pair AllReduce(add): every core contributes
its K/V tiles T2,T3 (tokens [s*2048+1024, (s+1)*2048)) multiplied by a
per-core host mask (1.0 on s=0 cores, 0.0 on s=1), so the pair-sum equals the
s=0 core's tiles = exactly the halo the s=1 core needs. s=0 cores receive
their own tiles back into the halo slots, which their kbias (-1e5 on jt<8)
already masks out — the program stays fully SPMD.

Schedule: KV+Q projections for tiles [2,3,0,1] (halo contribution first so the
collective overlaps the remaining projections), then attention+o_proj for
query blocks [2,3,0,1] (blocks 0,1 need the halo and run last).

DMA: sync queue carries the deadline-ordered input stream (wk/wv, x tiles,
rope tables, consts); gpsimd SW-DGE carries bulky weights (wq/tri/wo), the
collective in/out staging, and all o_part output writes (bf16; host sums).
"""

import sys

if "/opt/trn_rl_repo" not in sys.path:
    sys.path.insert(0, "/opt/trn_rl_repo")

import numpy as np

try:
    import ml_dtypes
    BF16 = ml_dtypes.bfloat16
except ImportError:
    BF16 = None

T, DM, NH, NKV, HD, WIN = 4096, 2048, 8, 4, 256, 1024
EPS, BASE = 1e-6, 10000.0
NG, NS = 4, 2
NQ = 2048
NTB = 4           # 512-token own-K/V tiles per core
NKO = 16          # 2048 / 128 contraction subtiles
SCALE = 1.0 / 16.0
NEG = -1.0e5

_cache = {}


def _host_prep(x, pos, Wq, Wk, Wv, Wo, q_norm_w, k_norm_w):
    x = np.asarray(x, np.float32).reshape(T, DM)
    xT = np.ascontiguousarray(x.T)
    pos_f = np.asarray(pos).astype(np.float64)
    m = np.arange(128)
    invf = BASE ** (-m / 128.0)

    Wq = np.asarray(Wq, np.float32)
    Wk = np.asarray(Wk, np.float32)
    Wv = np.asarray(Wv, np.float32)
    Wo = np.asarray(Wo, np.float32)
    qnw = np.asarray(q_norm_w, np.float32)
    knw = np.asarray(k_norm_w, np.float32)

    r0T = np.zeros((128, 128), np.float32)
    a = np.arange(64)
    r0T[2 * a, 2 * a + 1] = 1.0
    r0T[2 * a + 1, 2 * a] = -1.0
    qw2 = np.ascontiguousarray(np.stack([qnw[:128], qnw[128:]], axis=1))
    kw2 = np.ascontiguousarray(np.stack([knw[:128], knw[128:]], axis=1))

    # masks for 512-wide attention blocks: m=0..3 far edge, m=8..11 diagonal
    jp = np.arange(128)[:, None]
    ip = np.arange(512)[None, :]
    tris = []
    for mm_ in range(4):
        tris.append(jp >= ip + 1 - 128 * mm_)         # far masks F_m
    for mm_ in range(4):
        tris.append(jp <= ip - 128 * mm_)             # diag masks D_{m+8}
    tri = np.concatenate(tris, axis=1).astype(BF16)   # [128, 8*512]

    in_maps = []
    for g in range(NG):
        wkT = Wk[g * HD:(g + 1) * HD, :].T            # [DM, 256]
        wvT = Wv[g * HD:(g + 1) * HD, :].T
        wqT = Wq[2 * g * HD:(2 * g + 2) * HD, :].T    # [DM, 512]
        woT = Wo[:, 2 * g * HD:(2 * g + 2) * HD].T    # [512, DM]
        wkp = np.ascontiguousarray(
            wkT.reshape(NKO, 128, 256).transpose(1, 0, 2)).astype(BF16)
        wvp = np.ascontiguousarray(
            wvT.reshape(NKO, 128, 256).transpose(1, 0, 2)).astype(BF16)
        wqp = np.ascontiguousarray(
            wqT.reshape(NKO, 128, 512).transpose(1, 0, 2)).astype(BF16)
        wop = np.ascontiguousarray(
            woT.reshape(4, 128, DM).transpose(1, 0, 2)).astype(BF16)
        for s in range(NS):
            lo = s * 2048
            xT_c = xT[:, lo:lo + 2048]
            # [8, 128, 8, 512]: tile (t, half) -> ko = half*8..half*8+7
            xp = np.ascontiguousarray(
                xT_c.reshape(NKO, 128, NTB, 512)
                .transpose(2, 0, 1, 3)                 # [t, ko, p, tok]
                .reshape(NTB, 2, 8, 128, 512)
                .transpose(0, 1, 3, 2, 4)              # [t, half, p, k, tok]
                .reshape(NTB * 2, 128, 8, 512)).astype(BF16)

            p = pos_f[lo:lo + 2048]
            ang = p[None, :] * invf[:, None]
            cosk = np.cos(ang).astype(np.float32)      # [128, 2048]
            sink = np.sin(ang).astype(np.float32)
            cosp = np.ascontiguousarray(
                cosk.reshape(128, NTB, 512).transpose(1, 0, 2))
            sinp = np.ascontiguousarray(
                sink.reshape(128, NTB, 512).transpose(1, 0, 2))

            kbias = np.zeros((128, 24), np.float32)
            if s == 0:
                kbias[:, :8] = NEG
            msk = np.full((128, 1), 1.0 if s == 0 else 0.0, np.float32)

            in_maps.append({
                "xp": xp,
                "cosp": cosp,
                "sinp": sinp,
                "wkp": wkp,
                "wvp": wvp,
                "wqp": wqp,
                "wop": wop,
                "r0T": r0T.astype(BF16),
                "qw": qw2,
                "kw": kw2,
                "kbias": kbias,
                "msk": msk,
                "tri": tri,
            })
    return in_maps


def _build_program():
    if "nc" in _cache:
        return _cache["nc"]

    import concourse.bass as bass
    import concourse.mybir as mybir
    import concourse.tile as tile
    from concourse import bacc
    from contextlib import ExitStack

    f32 = mybir.dt.float32
    bf16 = mybir.dt.bfloat16
    AF = mybir.ActivationFunctionType
    OP = mybir.AluOpType

    nc = bacc.Bacc("TRN2", target_bir_lowering=False, debug=False,
                   enable_asserts=False, num_devices=8)

    xp_d = nc.dram_tensor("xp", [NTB * 2, 128, 8, 512], bf16, kind="ExternalInput")
    cosp_d = nc.dram_tensor("cosp", [NTB, 128, 512], f32, kind="ExternalInput")
    sinp_d = nc.dram_tensor("sinp", [NTB, 128, 512], f32, kind="ExternalInput")
    wq_d = nc.dram_tensor("wqp", [128, NKO, 512], bf16, kind="ExternalInput")
    wk_d = nc.dram_tensor("wkp", [128, NKO, 256], bf16, kind="ExternalInput")
    wv_d = nc.dram_tensor("wvp", [128, NKO, 256], bf16, kind="ExternalInput")
    wo_d = nc.dram_tensor("wop", [128, 4, DM], bf16, kind="ExternalInput")
    r0_d = nc.dram_tensor("r0T", [128, 128], bf16, kind="ExternalInput")
    qw_d = nc.dram_tensor("qw", [128, 2], f32, kind="ExternalInput")
    kw_d = nc.dram_tensor("kw", [128, 2], f32, kind="ExternalInput")
    kb_d = nc.dram_tensor("kbias", [128, 24], f32, kind="ExternalInput")
    msk_d = nc.dram_tensor("msk", [128, 1], f32, kind="ExternalInput")
    tri_d = nc.dram_tensor("tri", [128, 8 * 512], bf16, kind="ExternalInput")
    o_d = nc.dram_tensor("o_part", [NQ, DM], bf16, kind="ExternalOutput")

    cc_in = nc.dram_tensor("cc_in", [128, 4096], bf16, kind="Internal")
    cc_out = nc.dram_tensor("cc_out", [128, 4096], bf16, kind="Internal")

    with tile.TileContext(nc) as tc, ExitStack() as ctx:
        cpool = ctx.enter_context(tc.tile_pool(name="consts", bufs=1))
        xpool = ctx.enter_context(tc.tile_pool(name="xt", bufs=4))
        tabpool = ctx.enter_context(tc.tile_pool(name="tab", bufs=3))
        kvpool = ctx.enter_context(tc.tile_pool(name="kv", bufs=1))
        scpool = ctx.enter_context(tc.tile_pool(name="scratch", bufs=3))
        spool = ctx.enter_context(tc.tile_pool(name="small", bufs=2))
        qpool = ctx.enter_context(tc.tile_pool(name="qt", bufs=1))
        ptpool = ctx.enter_context(tc.tile_pool(name="pt", bufs=4))
        ypool = ctx.enter_context(tc.tile_pool(name="yt", bufs=2))
        opool = ctx.enter_context(tc.tile_pool(name="osb", bufs=3))
        pp_proj = ctx.enter_context(tc.tile_pool(name="pproj", bufs=3, space="PSUM"))
        pp_small = ctx.enter_context(tc.tile_pool(name="psmall", bufs=2, space="PSUM"))
        pp_acc = ctx.enter_context(tc.tile_pool(name="pacc", bufs=3, space="PSUM"))

        KVORD = [2, 3, 0, 1]      # projection tile order (halo contribution first)
        AORD = [2, 3, 0, 1]       # attention block order (halo consumers last)

        # ---- sync queue: deadline-ordered latency-critical stream ----
        wk_sb = cpool.tile([128, NKO, 256], bf16, tag="wk")
        nc.sync.dma_start(wk_sb[:, 0:8, :], wk_d.ap()[:, 0:8, :])
        xt_tiles = [None] * (NTB * 2)

        def load_x(t):
            for half in range(2):
                i = t * 2 + half
                xt = xpool.tile([128, 8, 512], bf16, tag="xt", name=f"xt_p{i}")
                nc.sync.dma_start(xt[:], xp_d.ap()[i])
                xt_tiles[i] = xt

        cos_tiles = [None] * NTB
        sin_tiles = [None] * NTB

        def load_tab(t):
            ct = tabpool.tile([128, 512], f32, tag="cos", name=f"cos{t}")
            nc.sync.dma_start(ct[:], cosp_d.ap()[t])
            st = tabpool.tile([128, 512], f32, tag="sin", name=f"sin{t}")
            nc.sync.dma_start(st[:], sinp_d.ap()[t])
            cos_tiles[t] = ct
            sin_tiles[t] = st

        load_x(KVORD[0])
        nc.sync.dma_start(wk_sb[:, 8:16, :], wk_d.ap()[:, 8:16, :])
        load_tab(KVORD[0])
        wq_sb = cpool.tile([128, NKO, 512], bf16, tag="wq")
        nc.sync.dma_start(wq_sb[:, 0:8, :], wq_d.ap()[:, 0:8, :])
        wv_sb = cpool.tile([128, NKO, 256], bf16, tag="wv")
        nc.sync.dma_start(wv_sb[:, 0:8, :], wv_d.ap()[:, 0:8, :])
        nc.sync.dma_start(wv_sb[:, 8:16, :], wv_d.ap()[:, 8:16, :])
        nc.sync.dma_start(wq_sb[:, 8:16, :], wq_d.ap()[:, 8:16, :])
        r0_sb = cpool.tile([128, 128], bf16, tag="r0")
        nc.sync.dma_start(r0_sb[:], r0_d.ap())
        qw_sb = cpool.tile([128, 2], f32, tag="qwt")
        nc.sync.dma_start(qw_sb[:], qw_d.ap())
        kw_sb = cpool.tile([128, 2], f32, tag="kwt")
        nc.sync.dma_start(kw_sb[:], kw_d.ap())
        kb_sb = cpool.tile([128, 24], f32, tag="kb")
        nc.sync.dma_start(kb_sb[:], kb_d.ap())
        msk_sb = cpool.tile([128, 1], f32, tag="msk")
        nc.sync.dma_start(msk_sb[:], msk_d.ap())
        load_x(KVORD[1])
        load_tab(KVORD[1])
        load_x(KVORD[2])
        load_tab(KVORD[2])
        load_x(KVORD[3])
        load_tab(KVORD[3])

        # ---- gpsimd queue: bulky latency-tolerant weights ----
        tri_sb = cpool.tile([128, 8 * 512], bf16, tag="tri")
        nc.gpsimd.dma_start(tri_sb[:], tri_d.ap())
        wo_sb = cpool.tile([128, 4, DM], bf16, tag="wo")
        nc.gpsimd.dma_start(wo_sb[:, 0:2, :], wo_d.ap()[:, 0:2, :])
        nc.gpsimd.dma_start(wo_sb[:, 2:4, :], wo_d.ap()[:, 2:4, :])

        ones_sb = cpool.tile([128, 128], bf16, tag="ones")
        nc.vector.memset(ones_sb[:], 1.0)
        eps_sb = cpool.tile([128, 1], f32, tag="eps")
        nc.vector.memset(eps_sb[:], EPS)
        zero_sb = cpool.tile([128, 1], f32, tag="zero")
        nc.vector.memset(zero_sb[:], 0.0)

        # K/V slots: 0,1 = halo (filled by the collective), 2..5 = own tiles
        kt_tiles = []
        vt_tiles = []
        for sl in range(6):
            ktile = kvpool.tile([128, 2, 512], bf16, tag=f"kt{sl}", name=f"kt{sl}")
            kt_tiles.append(ktile)
            vtile = kvpool.tile([128, 4, 256], bf16, tag=f"vt{sl}", name=f"vt{sl}")
            vt_tiles.append(vtile)
        qt_tiles = [None] * NTB

        def norm_rope(src_ps, w_sb, cos_t, sin_t, dst, dsti):
            z2 = scpool.tile([128, 2, 512], bf16, tag="z2")
            for u in range(2):
                nc.scalar.activation(z2[:, u, :], src_ps[u][:], AF.Square,
                                     bias=zero_sb[:])
            ssq = pp_small.tile([128, 512], f32, tag="psm")
            for u in range(2):
                nc.tensor.matmul(ssq[:], ones_sb[:], z2[:, u, :],
                                 start=(u == 0), stop=(u == 1))
            sq = spool.tile([128, 512], f32, tag="sq")
            nc.scalar.activation(sq[:], ssq[:], AF.Sqrt, bias=eps_sb[:], scale=1.0 / HD)
            rs = spool.tile([128, 512], f32, tag="rs")
            nc.vector.reciprocal_approx_fast(rs[:], sq[:])
            znw = scpool.tile([128, 2, 512], bf16, tag="znw")
            t1 = scpool.tile([128, 2, 512], f32, tag="t1")
            for u in range(2):
                nc.vector.scalar_tensor_tensor(
                    znw[:, u, :], src_ps[u][:], w_sb[:, u:u + 1], rs[:],
                    OP.mult, OP.mult)
                rot = pp_small.tile([128, 512], f32, tag="psm")
                nc.tensor.matmul(rot[:], r0_sb[:], znw[:, u, :], start=True, stop=True)
                nc.vector.tensor_tensor(t1[:, u, :], znw[:, u, :], cos_t, OP.mult)
                tmp = spool.tile([128, 512], f32, tag="tmp")
                nc.vector.tensor_tensor(tmp[:], rot[:], sin_t, OP.mult)
                nc.vector.tensor_tensor(dst[:, dsti + u, :], t1[:, u, :], tmp[:], OP.add)

        # ---- projection phase: tiles in KVORD; Q right after its K/V ----
        for t in KVORD:
            xth = xt_tiles[t * 2:t * 2 + 2]
            cos_t = cos_tiles[t]
            sin_t = sin_tiles[t]
            sl = t + 2            # own tile t lands in halo-space slot t+2

            # K projection (transposed, N=512), ko-inner
            k0_ps = pp_proj.tile([128, 512], f32, tag="pj")
            k1_ps = pp_proj.tile([128, 512], f32, tag="pj")
            k_ps = [k0_ps, k1_ps]
            for ko in range(NKO):
                for dsub in range(2):
                    nc.tensor.matmul(k_ps[dsub][:],
                                     wk_sb[:, ko, dsub * 128:(dsub + 1) * 128],
                                     xth[ko // 8][:, ko % 8, :],
                                     start=(ko == 0), stop=(ko == NKO - 1),
                                     skip_group_check=True)
            norm_rope(k_ps, kw_sb, cos_t[:], sin_t[:], kt_tiles[sl], 0)

            # Q projections (2 heads, N=512)
            qt_sb = qpool.tile([128, 4, 512], bf16, tag=f"q{t}", name=f"qt{t}")
            qt_tiles[t] = qt_sb
            for h in range(2):
                q0_ps = pp_proj.tile([128, 512], f32, tag="pj")
                q1_ps = pp_proj.tile([128, 512], f32, tag="pj")
                q_ps = [q0_ps, q1_ps]
                for u in range(2):
                    dsub = 2 * h + u
                    for ko in range(NKO):
                        nc.tensor.matmul(q_ps[u][:],
                                         wq_sb[:, ko, dsub * 128:(dsub + 1) * 128],
                                         xth[ko // 8][:, ko % 8, :],
                                         start=(ko == 0), stop=(ko == NKO - 1))
                norm_rope(q_ps, qw_sb, cos_t[:], sin_t[:], qt_sb, 2 * h)

            # V projection (natural layout); released fast by the copies, so
            # the next tile's first PSUM allocations never stall
            vt = vt_tiles[sl]
            for vh in range(2):
                v_ps = pp_proj.tile([128, 2, 256], f32, tag="pj")
                for ms in range(2):
                    msub = vh * 2 + ms
                    for ko in range(NKO):
                        nc.tensor.matmul(v_ps[:, ms, :],
                                         xth[ko // 8][:, ko % 8, msub * 128:(msub + 1) * 128],
                                         wv_sb[:, ko, :],
                                         start=(ko == 0), stop=(ko == NKO - 1))
                for ms in range(2):
                    nc.vector.tensor_copy(vt[:, vh * 2 + ms, :], v_ps[:, ms, :])

            if t == KVORD[1]:
                # halo contribution: tiles T2,T3 (slots 4,5) masked by msk
                contrib = cpool.tile([128, 4, 1024], bf16, tag="contrib")
                nc.vector.tensor_scalar(
                    contrib[:, 0, :], kt_tiles[4][:].rearrange("p a b -> p (a b)"),
                    msk_sb[:], None, OP.mult)
                nc.vector.tensor_scalar(
                    contrib[:, 1, :], kt_tiles[5][:].rearrange("p a b -> p (a b)"),
                    msk_sb[:], None, OP.mult)
                nc.vector.tensor_scalar(
                    contrib[:, 2, :], vt_tiles[4][:].rearrange("p a b -> p (a b)"),
                    msk_sb[:], None, OP.mult)
                nc.vector.tensor_scalar(
                    contrib[:, 3, :], vt_tiles[5][:].rearrange("p a b -> p (a b)"),
                    msk_sb[:], None, OP.mult)
                nc.sync.dma_start(cc_in.ap(), contrib[:])
                nc.gpsimd.collective_compute(
                    "AllReduce", OP.add,
                    replica_groups=[[0, 1], [2, 3], [4, 5], [6, 7]],
                    ins=[cc_in.ap()],
                    outs=[cc_out.ap()],
                )
                nc.sync.dma_start(kt_tiles[0][:], cc_out.ap()[:, 0:1024])
                nc.sync.dma_start(kt_tiles[1][:], cc_out.ap()[:, 1024:2048])
                nc.sync.dma_start(vt_tiles[0][:], cc_out.ap()[:, 2048:3072])
                nc.sync.dma_start(vt_tiles[1][:], cc_out.ap()[:, 3072:4096])

        # ---- attention + o-projection per 512-query block ----
        for a in AORD:
            qt_sb = qt_tiles[a]
            yt_sb = ypool.tile([128, 4, 512], bf16, tag="y")
            for h in range(2):
                dn_ps = pp_acc.tile([128, 512], f32, tag="pac")
                y0_ps = pp_acc.tile([128, 512], f32, tag="pac")
                y1_ps = pp_acc.tile([128, 512], f32, tag="pac")
                y_ps = [y0_ps, y1_ps]
                for mi, mrel in enumerate([3, 0, 1, 2] + list(range(4, 12))):
                    jt = 4 * a + mrel
                    ct, jh = jt // 4, jt % 4
                    ktc = kt_tiles[ct]
                    vtc = vt_tiles[ct]
                    if mrel <= 2:
                        ia, ib = 0, 128 * (mrel + 1)
                    elif mrel >= 9:
                        ia, ib = 128 * (mrel - 8), 512
                    else:
                        ia, ib = 0, 512
                    pt = ptpool.tile([128, 512], bf16, tag="p")
                    st = pp_small.tile([128, 512], f32, tag="psm")
                    for u in range(2):
                        nc.tensor.matmul(st[:, ia:ib],
                                         ktc[:, u, jh * 128:(jh + 1) * 128],
                                         qt_sb[:, 2 * h + u, ia:ib],
                                         start=(u == 0), stop=(u == 1))
                    nc.scalar.activation(pt[:, ia:ib], st[:, ia:ib], AF.Exp,
                                         bias=kb_sb[:, jt:jt + 1], scale=SCALE)
                    if mrel < 4:
                        nc.vector.tensor_tensor(
                            pt[:, ia:ib], pt[:, ia:ib],
                            tri_sb[:, mrel * 512 + ia:mrel * 512 + ib], OP.mult)
                    elif mrel >= 8:
                        nc.vector.tensor_tensor(
                            pt[:, ia:ib], pt[:, ia:ib],
                            tri_sb[:, (mrel - 4) * 512 + ia:(mrel - 4) * 512 + ib],
                            OP.mult)
                    first, last = (mi == 0), (mrel == 11)
                    nc.tensor.matmul(dn_ps[:, ia:ib], ones_sb[:], pt[:, ia:ib],
                                     start=first, stop=last, skip_group_check=True)
                    for dh in range(2):
                        nc.tensor.matmul(y_ps[dh][:, ia:ib],
                                         vtc[:, jh, dh * 128:(dh + 1) * 128],
                                         pt[:, ia:ib], start=first, stop=last,
                                         skip_group_check=True)
                rc = spool.tile([128, 512], f32, tag="rc")
                nc.vector.reciprocal_approx_fast(rc[:], dn_ps[:])
                for dh in range(2):
                    nc.vector.tensor_tensor(yt_sb[:, 2 * h + dh, :],
                                            y_ps[dh][:], rc[:], OP.mult)

            for msub in range(4):
                for dmh in range(2):
                    o_sb = opool.tile([128, 1024], bf16, tag="o")
                    for dq in range(2):
                        c0 = (dmh * 2 + dq) * 512
                        o_ps = pp_small.tile([128, 512], f32, tag="psm")
                        for hd in range(4):
                            nc.tensor.matmul(o_ps[:],
                                             yt_sb[:, hd, msub * 128:(msub + 1) * 128],
                                             wo_sb[:, hd, c0:c0 + 512],
                                             start=(hd == 0), stop=(hd == 3))
                        nc.scalar.copy(o_sb[:, dq * 512:(dq + 1) * 512], o_ps[:])
                    r0_ = a * 512 + msub * 128
                    nc.scalar.dma_start(o_d.ap()[r0_:r0_ + 128, dmh * 1024:(dmh + 1) * 1024],
                                        o_sb[:])

    nc.compile()
    _cache["nc"] = nc
    return nc


def _run(inputs, trace=False):
    from concourse.bass_utils import run_bass_kernel_spmd

    nc = _build_program()
    in_maps = _host_prep(**inputs)
    res = run_bass_kernel_spmd(nc, in_maps, core_ids=list(range(8)), trace=trace)
    full = np.zeros((T, DM), np.float32)
    for g in range(NG):
        for s in range(NS):
            full[s * 2048:(s + 1) * 2048] += res.results[g * 2 + s]["o_part"].astype(np.float32)
    return full.reshape(1, T, DM), res


def kernel(**inputs):
    return _run(inputs, trace=False)[0]


# revision 10
# speedup vs baseline: 1.1105x; 1.0382x over previous
"""Trainium2 Bass kernel for Gemma3 sliding-window attention. v2: halo exchange.

Sharding (8 cores): 4 KV-head groups x 2 sequence halves. Core (g, s) computes
query heads {2g, 2g+1} and KV head g for query tokens [s*2048, (s+1)*2048).
Unlike v1 (which recomputed a 1024-token KV halo locally from zero-padded x),
each core projects K/V only for its OWN 2048 tokens; the halo K/V for the
second-half cores is delivered by a pair AllReduce(add): every core contributes
its K/V tiles T2,T3 (tokens [s*2048+1024, (s+1)*2048)) multiplied by a
per-core host mask (1.0 on s=0 cores, 0.0 on s=1), so the pair-sum equals the
s=0 core's tiles = exactly the halo the s=1 core needs. s=0 cores receive
their own tiles back into the halo slots, which their kbias (-1e5 on jt<8)
already masks out — the program stays fully SPMD.

Schedule: KV+Q projections for tiles [2,3,0,1] (halo contribution first so the
collective overlaps the remaining projections), then attention+o_proj for
query blocks [2,3,0,1] (blocks 0,1 need the halo and run last).

DMA: sync queue carries the deadline-ordered input stream (wk/wv, x tiles,
rope tables, consts); gpsimd SW-DGE carries bulky weights (wq/tri/wo), the
collective in/out staging, and all o_part output writes (bf16; host sums).
"""

import sys

if "/opt/trn_rl_repo" not in sys.path:
    sys.path.insert(0, "/opt/trn_rl_repo")

import numpy as np

try:
    import ml_dtypes
    BF16 = ml_dtypes.bfloat16
except ImportError:
    BF16 = None

T, DM, NH, NKV, HD, WIN = 4096, 2048, 8, 4, 256, 1024
EPS, BASE = 1e-6, 10000.0
NG, NS = 4, 2
NQ = 2048
NTB = 4           # 512-token own-K/V tiles per core
NKO = 16          # 2048 / 128 contraction subtiles
SCALE = 1.0 / 16.0
NEG = -1.0e5

_cache = {}


def _host_prep(x, pos, Wq, Wk, Wv, Wo, q_norm_w, k_norm_w):
    x = np.asarray(x, np.float32).reshape(T, DM)
    xT = np.ascontiguousarray(x.T)
    pos_f = np.asarray(pos).astype(np.float64)
    m = np.arange(128)
    invf = BASE ** (-m / 128.0)

    Wq = np.asarray(Wq, np.float32)
    Wk = np.asarray(Wk, np.float32)
    Wv = np.asarray(Wv, np.float32)
    Wo = np.asarray(Wo, np.float32)
    qnw = np.asarray(q_norm_w, np.float32)
    knw = np.asarray(k_norm_w, np.float32)

    r0T = np.zeros((128, 128), np.float32)
    a = np.arange(64)
    r0T[2 * a, 2 * a + 1] = 1.0
    r0T[2 * a + 1, 2 * a] = -1.0
    qw2 = np.ascontiguousarray(np.stack([qnw[:128], qnw[128:]], axis=1))
    kw2 = np.ascontiguousarray(np.stack([knw[:128], knw[128:]], axis=1))

    # masks for 512-wide attention blocks: m=0..3 far edge, m=8..11 diagonal
    jp = np.arange(128)[:, None]
    ip = np.arange(512)[None, :]
    tris = []
    for mm_ in range(4):
        tris.append(jp >= ip + 1 - 128 * mm_)         # far masks F_m
    for mm_ in range(4):
        tris.append(jp <= ip - 128 * mm_)             # diag masks D_{m+8}
    tri = np.concatenate(tris, axis=1).astype(BF16)   # [128, 8*512]

    in_maps = []
    for g in range(NG):
        wkT = Wk[g * HD:(g + 1) * HD, :].T            # [DM, 256]
        wvT = Wv[g * HD:(g + 1) * HD, :].T
        wqT = Wq[2 * g * HD:(2 * g + 2) * HD, :].T    # [DM, 512]
        woT = Wo[:, 2 * g * HD:(2 * g + 2) * HD].T    # [512, DM]
        wkp = np.ascontiguousarray(
            wkT.reshape(NKO, 128, 256).transpose(1, 0, 2)).astype(BF16)
        wvp = np.ascontiguousarray(
            wvT.reshape(NKO, 128, 256).transpose(1, 0, 2)).astype(BF16)
        wqp = np.ascontiguousarray(
            wqT.reshape(NKO, 128, 512).transpose(1, 0, 2)).astype(BF16)
        wop = np.ascontiguousarray(
            woT.reshape(4, 128, DM).transpose(1, 0, 2)).astype(BF16)
        for s in range(NS):
            lo = s * 2048
            xT_c = xT[:, lo:lo + 2048]
            # [8, 128, 8, 512]: tile (t, half) -> ko = half*8..half*8+7
            xp = np.ascontiguousarray(
                xT_c.reshape(NKO, 128, NTB, 512)
                .transpose(2, 0, 1, 3)                 # [t, ko, p, tok]
                .reshape(NTB, 2, 8, 128, 512)
                .transpose(0, 1, 3, 2, 4)              # [t, half, p, k, tok]
                .reshape(NTB * 2, 128, 8, 512)).astype(BF16)

            p = pos_f[lo:lo + 2048]
            ang = p[None, :] * invf[:, None]
            cosk = np.cos(ang).astype(np.float32)      # [128, 2048]
            sink = np.sin(ang).astype(np.float32)
            cosp = np.ascontiguousarray(
                cosk.reshape(128, NTB, 512).transpose(1, 0, 2))
            sinp = np.ascontiguousarray(
                sink.reshape(128, NTB, 512).transpose(1, 0, 2))

            kbias = np.zeros((128, 24), np.float32)
            if s == 0:
                kbias[:, :8] = NEG
            msk = np.full((128, 1), 1.0 if s == 0 else 0.0, np.float32)

            in_maps.append({
                "xp": xp,
                "cosp": cosp,
                "sinp": sinp,
                "wkp": wkp,
                "wvp": wvp,
                "wqp": wqp,
                "wop": wop,
                "r0T": r0T.astype(BF16),
                "qw": qw2,
                "kw": kw2,
                "kbias": kbias,
                "msk": msk,
                "tri": tri,
            })
    return in_maps


def _build_program():
    if "nc" in _cache:
        return _cache["nc"]

    import concourse.bass as bass
    import concourse.mybir as mybir
    import concourse.tile as tile
    from concourse import bacc
    from contextlib import ExitStack

    f32 = mybir.dt.float32
    bf16 = mybir.dt.bfloat16
    AF = mybir.ActivationFunctionType
    OP = mybir.AluOpType

    nc = bacc.Bacc("TRN2", target_bir_lowering=False, debug=False,
                   enable_asserts=False, num_devices=8)

    xp_d = nc.dram_tensor("xp", [NTB * 2, 128, 8, 512], bf16, kind="ExternalInput")
    cosp_d = nc.dram_tensor("cosp", [NTB, 128, 512], f32, kind="ExternalInput")
    sinp_d = nc.dram_tensor("sinp", [NTB, 128, 512], f32, kind="ExternalInput")
    wq_d = nc.dram_tensor("wqp", [128, NKO, 512], bf16, kind="ExternalInput")
    wk_d = nc.dram_tensor("wkp", [128, NKO, 256], bf16, kind="ExternalInput")
    wv_d = nc.dram_tensor("wvp", [128, NKO, 256], bf16, kind="ExternalInput")
    wo_d = nc.dram_tensor("wop", [128, 4, DM], bf16, kind="ExternalInput")
    r0_d = nc.dram_tensor("r0T", [128, 128], bf16, kind="ExternalInput")
    qw_d = nc.dram_tensor("qw", [128, 2], f32, kind="ExternalInput")
    kw_d = nc.dram_tensor("kw", [128, 2], f32, kind="ExternalInput")
    kb_d = nc.dram_tensor("kbias", [128, 24], f32, kind="ExternalInput")
    msk_d = nc.dram_tensor("msk", [128, 1], f32, kind="ExternalInput")
    tri_d = nc.dram_tensor("tri", [128, 8 * 512], bf16, kind="ExternalInput")
    o_d = nc.dram_tensor("o_part", [NQ, DM], bf16, kind="ExternalOutput")

    cc_in = nc.dram_tensor("cc_in", [128, 4096], bf16, kind="Internal")
    cc_out = nc.dram_tensor("cc_out", [128, 4096], bf16, kind="Internal")

    with tile.TileContext(nc) as tc, ExitStack() as ctx:
        cpool = ctx.enter_context(tc.tile_pool(name="consts", bufs=1))
        xpool = ctx.enter_context(tc.tile_pool(name="xt", bufs=4))
        tabpool = ctx.enter_context(tc.tile_pool(name="tab", bufs=3))
        kvpool = ctx.enter_context(tc.tile_pool(name="kv", bufs=1))
        scpool = ctx.enter_context(tc.tile_pool(name="scratch", bufs=3))
        spool = ctx.enter_context(tc.tile_pool(name="small", bufs=2))
        qpool = ctx.enter_context(tc.tile_pool(name="qt", bufs=1))
        ptpool = ctx.enter_context(tc.tile_pool(name="pt", bufs=4))
        ypool = ctx.enter_context(tc.tile_pool(name="yt", bufs=2))
        opool = ctx.enter_context(tc.tile_pool(name="osb", bufs=3))
        pp_small = ctx.enter_context(tc.tile_pool(name="psmall", bufs=2, space="PSUM"))
        proj_ctx = ExitStack()
        pp_proj = proj_ctx.enter_context(tc.tile_pool(name="pproj", bufs=3, space="PSUM"))

        KVORD = [2, 3, 0, 1]      # projection tile order (halo contribution first)
        AORD = [2, 3, 0, 1]       # attention block order (halo consumers last)

        # ---- sync queue: deadline-ordered latency-critical stream ----
        wk_sb = cpool.tile([128, NKO, 256], bf16, tag="wk")
        nc.sync.dma_start(wk_sb[:, 0:8, :], wk_d.ap()[:, 0:8, :])
        xt_tiles = [None] * (NTB * 2)

        def load_x(t):
            for half in range(2):
                i = t * 2 + half
                xt = xpool.tile([128, 8, 512], bf16, tag="xt", name=f"xt_p{i}")
                nc.sync.dma_start(xt[:], xp_d.ap()[i])
                xt_tiles[i] = xt

        cos_tiles = [None] * NTB
        sin_tiles = [None] * NTB

        def load_tab(t):
            ct = tabpool.tile([128, 512], f32, tag="cos", name=f"cos{t}")
            nc.sync.dma_start(ct[:], cosp_d.ap()[t])
            st = tabpool.tile([128, 512], f32, tag="sin", name=f"sin{t}")
            nc.sync.dma_start(st[:], sinp_d.ap()[t])
            cos_tiles[t] = ct
            sin_tiles[t] = st

        load_x(KVORD[0])
        nc.sync.dma_start(wk_sb[:, 8:16, :], wk_d.ap()[:, 8:16, :])
        load_tab(KVORD[0])
        wq_sb = cpool.tile([128, NKO, 512], bf16, tag="wq")
        nc.sync.dma_start(wq_sb[:, 0:8, :], wq_d.ap()[:, 0:8, :])
        nc.sync.dma_start(wq_sb[:, 8:16, :], wq_d.ap()[:, 8:16, :])
        r0_sb = cpool.tile([128, 128], bf16, tag="r0")
        nc.sync.dma_start(r0_sb[:], r0_d.ap())
        qw_sb = cpool.tile([128, 2], f32, tag="qwt")
        nc.sync.dma_start(qw_sb[:], qw_d.ap())
        kw_sb = cpool.tile([128, 2], f32, tag="kwt")
        nc.sync.dma_start(kw_sb[:], kw_d.ap())
        kb_sb = cpool.tile([128, 24], f32, tag="kb")
        nc.sync.dma_start(kb_sb[:], kb_d.ap())
        msk_sb = cpool.tile([128, 1], f32, tag="msk")
        nc.sync.dma_start(msk_sb[:], msk_d.ap())
        wv_sb = cpool.tile([128, NKO, 256], bf16, tag="wv")
        nc.sync.dma_start(wv_sb[:, 0:8, :], wv_d.ap()[:, 0:8, :])
        nc.sync.dma_start(wv_sb[:, 8:16, :], wv_d.ap()[:, 8:16, :])
        load_x(KVORD[1])
        load_tab(KVORD[1])
        load_x(KVORD[2])
        load_tab(KVORD[2])
        load_x(KVORD[3])
        load_tab(KVORD[3])
        # latency-tolerant weights stream after the proj-phase inputs
        tri_sb = cpool.tile([128, 8 * 512], bf16, tag="tri")
        nc.sync.dma_start(tri_sb[:], tri_d.ap())
        wo_sb = cpool.tile([128, 4, DM], bf16, tag="wo")
        nc.sync.dma_start(wo_sb[:, 0:2, :], wo_d.ap()[:, 0:2, :])
        nc.sync.dma_start(wo_sb[:, 2:4, :], wo_d.ap()[:, 2:4, :])

        ones_sb = cpool.tile([128, 128], bf16, tag="ones")
        nc.vector.memset(ones_sb[:], 1.0)
        eps_sb = cpool.tile([128, 1], f32, tag="eps")
        nc.vector.memset(eps_sb[:], EPS)
        zero_sb = cpool.tile([128, 1], f32, tag="zero")
        nc.vector.memset(zero_sb[:], 0.0)

        # K/V slots: 0,1 = halo (filled by the collective), 2..5 = own tiles
        kt_tiles = []
        vt_tiles = []
        for sl in range(6):
            ktile = kvpool.tile([128, 2, 512], bf16, tag=f"kt{sl}", name=f"kt{sl}")
            kt_tiles.append(ktile)
            vtile = kvpool.tile([128, 4, 256], bf16, tag=f"vt{sl}", name=f"vt{sl}")
            vt_tiles.append(vtile)
        qt_tiles = [None] * NTB

        def norm_rope(src_ps, w_sb, cos_t, sin_t, dst, dsti):
            z2 = scpool.tile([128, 2, 512], bf16, tag="z2")
            for u in range(2):
                nc.scalar.activation(z2[:, u, :], src_ps[u][:], AF.Square,
                                     bias=zero_sb[:])
            ssq = pp_small.tile([128, 512], f32, tag="psm")
            for u in range(2):
                nc.tensor.matmul(ssq[:], ones_sb[:], z2[:, u, :],
                                 start=(u == 0), stop=(u == 1))
            sq = spool.tile([128, 512], f32, tag="sq")
            nc.scalar.activation(sq[:], ssq[:], AF.Sqrt, bias=eps_sb[:], scale=1.0 / HD)
            rs = spool.tile([128, 512], f32, tag="rs")
            nc.vector.reciprocal_approx_fast(rs[:], sq[:])
            znw = scpool.tile([128, 2, 512], bf16, tag="znw")
            t1 = scpool.tile([128, 2, 512], f32, tag="t1")
            for u in range(2):
                nc.vector.scalar_tensor_tensor(
                    znw[:, u, :], src_ps[u][:], w_sb[:, u:u + 1], rs[:],
                    OP.mult, OP.mult)
                rot = pp_small.tile([128, 512], f32, tag="psm")
                nc.tensor.matmul(rot[:], r0_sb[:], znw[:, u, :], start=True, stop=True)
                nc.vector.tensor_tensor(t1[:, u, :], znw[:, u, :], cos_t, OP.mult)
                tmp = spool.tile([128, 512], f32, tag="tmp")
                nc.vector.tensor_tensor(tmp[:], rot[:], sin_t, OP.mult)
                nc.vector.tensor_tensor(dst[:, dsti + u, :], t1[:, u, :], tmp[:], OP.add)

        # ---- projection phase: tiles in KVORD; Q right after its K/V ----
        for t in KVORD:
            xth = xt_tiles[t * 2:t * 2 + 2]
            cos_t = cos_tiles[t]
            sin_t = sin_tiles[t]
            sl = t + 2            # own tile t lands in halo-space slot t+2

            # K projection (transposed, N=512), ko-inner
            k0_ps = pp_proj.tile([128, 512], f32, tag="pj")
            k1_ps = pp_proj.tile([128, 512], f32, tag="pj")
            k_ps = [k0_ps, k1_ps]
            for ko in range(NKO):
                for dsub in range(2):
                    nc.tensor.matmul(k_ps[dsub][:],
                                     wk_sb[:, ko, dsub * 128:(dsub + 1) * 128],
                                     xth[ko // 8][:, ko % 8, :],
                                     start=(ko == 0), stop=(ko == NKO - 1),
                                     skip_group_check=True)
            norm_rope(k_ps, kw_sb, cos_t[:], sin_t[:], kt_tiles[sl], 0)

            # Q projections (2 heads, N=512)
            qt_sb = qpool.tile([128, 4, 512], bf16, tag=f"q{t}", name=f"qt{t}")
            qt_tiles[t] = qt_sb
            for h in range(2):
                q0_ps = pp_proj.tile([128, 512], f32, tag="pj")
                q1_ps = pp_proj.tile([128, 512], f32, tag="pj")
                q_ps = [q0_ps, q1_ps]
                for u in range(2):
                    dsub = 2 * h + u
                    for ko in range(NKO):
                        nc.tensor.matmul(q_ps[u][:],
                                         wq_sb[:, ko, dsub * 128:(dsub + 1) * 128],
                                         xth[ko // 8][:, ko % 8, :],
                                         start=(ko == 0), stop=(ko == NKO - 1))
                norm_rope(q_ps, qw_sb, cos_t[:], sin_t[:], qt_sb, 2 * h)

            # V projection (natural layout); released fast by the copies, so
            # the next tile's first PSUM allocations never stall
            vt = vt_tiles[sl]
            for vh in range(2):
                v_ps = pp_proj.tile([128, 2, 256], f32, tag="pj")
                for ms in range(2):
                    msub = vh * 2 + ms
                    for ko in range(NKO):
                        nc.tensor.matmul(v_ps[:, ms, :],
                                         xth[ko // 8][:, ko % 8, msub * 128:(msub + 1) * 128],
                                         wv_sb[:, ko, :],
                                         start=(ko == 0), stop=(ko == NKO - 1))
                for ms in range(2):
                    nc.vector.tensor_copy(vt[:, vh * 2 + ms, :], v_ps[:, ms, :])

            if t == KVORD[1]:
                # halo contribution: tiles T2,T3 (slots 4,5) masked by msk
                contrib = cpool.tile([128, 4, 1024], bf16, tag="contrib")
                nc.vector.tensor_scalar(
                    contrib[:, 0, :], kt_tiles[4][:].rearrange("p a b -> p (a b)"),
                    msk_sb[:], None, OP.mult)
                nc.vector.tensor_scalar(
                    contrib[:, 1, :], kt_tiles[5][:].rearrange("p a b -> p (a b)"),
                    msk_sb[:], None, OP.mult)
                nc.vector.tensor_scalar(
                    contrib[:, 2, :], vt_tiles[4][:].rearrange("p a b -> p (a b)"),
                    msk_sb[:], None, OP.mult)
                nc.vector.tensor_scalar(
                    contrib[:, 3, :], vt_tiles[5][:].rearrange("p a b -> p (a b)"),
                    msk_sb[:], None, OP.mult)
                nc.sync.dma_start(cc_in.ap(), contrib[:])
                nc.gpsimd.collective_compute(
                    "AllReduce", OP.add,
                    replica_groups=[[0, 1], [2, 3], [4, 5], [6, 7]],
                    ins=[cc_in.ap()],
                    outs=[cc_out.ap()],
                )
                nc.sync.dma_start(kt_tiles[0][:], cc_out.ap()[:, 0:1024])
                nc.sync.dma_start(kt_tiles[1][:], cc_out.ap()[:, 1024:2048])
                nc.sync.dma_start(vt_tiles[0][:], cc_out.ap()[:, 2048:3072])
                nc.sync.dma_start(vt_tiles[1][:], cc_out.ap()[:, 3072:4096])

        proj_ctx.close()
        pp_acc = ctx.enter_context(tc.tile_pool(name="pacc", bufs=6, space="PSUM"))

        # ---- attention + o-projection per 512-query block ----
        for a in AORD:
            qt_sb = qt_tiles[a]
            yt_sb = ypool.tile([128, 4, 512], bf16, tag="y")
            for h in range(2):
                dn_ps = pp_acc.tile([128, 512], f32, tag="pac")
                y0_ps = pp_acc.tile([128, 512], f32, tag="pac")
                y1_ps = pp_acc.tile([128, 512], f32, tag="pac")
                y_ps = [y0_ps, y1_ps]
                for mi, mrel in enumerate([3, 0, 1, 2] + list(range(4, 12))):
                    jt = 4 * a + mrel
                    ct, jh = jt // 4, jt % 4
                    ktc = kt_tiles[ct]
                    vtc = vt_tiles[ct]
                    if mrel <= 2:
                        ia, ib = 0, 128 * (mrel + 1)
                    elif mrel >= 9:
                        ia, ib = 128 * (mrel - 8), 512
                    else:
                        ia, ib = 0, 512
                    pt = ptpool.tile([128, 512], bf16, tag="p")
                    st = pp_small.tile([128, 512], f32, tag="psm")
                    for u in range(2):
                        nc.tensor.matmul(st[:, ia:ib],
                                         ktc[:, u, jh * 128:(jh + 1) * 128],
                                         qt_sb[:, 2 * h + u, ia:ib],
                                         start=(u == 0), stop=(u == 1))
                    nc.scalar.activation(pt[:, ia:ib], st[:, ia:ib], AF.Exp,
                                         bias=kb_sb[:, jt:jt + 1], scale=SCALE)
                    if mrel < 4:
                        nc.vector.tensor_tensor(
                            pt[:, ia:ib], pt[:, ia:ib],
                            tri_sb[:, mrel * 512 + ia:mrel * 512 + ib], OP.mult)
                    elif mrel >= 8:
                        nc.vector.tensor_tensor(
                            pt[:, ia:ib], pt[:, ia:ib],
                            tri_sb[:, (mrel - 4) * 512 + ia:(mrel - 4) * 512 + ib],
                            OP.mult)
                    first, last = (mi == 0), (mrel == 11)
                    nc.tensor.matmul(dn_ps[:, ia:ib], ones_sb[:], pt[:, ia:ib],
                                     start=first, stop=last, skip_group_check=True)
                    for dh in range(2):
                        nc.tensor.matmul(y_ps[dh][:, ia:ib],
                                         vtc[:, jh, dh * 128:(dh + 1) * 128],
                                         pt[:, ia:ib], start=first, stop=last,
                                         skip_group_check=True)
                rc = spool.tile([128, 512], f32, tag="rc")
                nc.vector.reciprocal_approx_fast(rc[:], dn_ps[:])
                for dh in range(2):
                    nc.vector.tensor_tensor(yt_sb[:, 2 * h + dh, :],
                                            y_ps[dh][:], rc[:], OP.mult)

            for msub in range(4):
                for dmh in range(2):
                    o_sb = opool.tile([128, 1024], bf16, tag="o")
                    for dq in range(2):
                        c0 = (dmh * 2 + dq) * 512
                        o_ps = pp_small.tile([128, 512], f32, tag="psm")
                        for hd in range(4):
                            nc.tensor.matmul(o_ps[:],
                                             yt_sb[:, hd, msub * 128:(msub + 1) * 128],
                                             wo_sb[:, hd, c0:c0 + 512],
                                             start=(hd == 0), stop=(hd == 3))
                        nc.scalar.copy(o_sb[:, dq * 512:(dq + 1) * 512], o_ps[:])
                    r0_ = a * 512 + msub * 128
                    nc.scalar.dma_start(o_d.ap()[r0_:r0_ + 128, dmh * 1024:(dmh + 1) * 1024],
                                        o_sb[:])

    nc.compile()
    _cache["nc"] = nc
    return nc


def _run(inputs, trace=False):
    from concourse.bass_utils import run_bass_kernel_spmd

    nc = _build_program()
    in_maps = _host_prep(**inputs)
    res = run_bass_kernel_spmd(nc, in_maps, core_ids=list(range(8)), trace=trace)
    full = np.zeros((T, DM), np.float32)
    for g in range(NG):
        for s in range(NS):
            full[s * 2048:(s + 1) * 2048] += res.results[g * 2 + s]["o_part"].astype(np.float32)
    return full.reshape(1, T, DM), res


def kernel(**inputs):
    return _run(inputs, trace=False)[0]


# revision 11
# speedup vs baseline: 1.1424x; 1.0288x over previous
"""Trainium2 Bass kernel for Gemma3 sliding-window attention. v2: halo exchange.

Sharding (8 cores): 4 KV-head groups x 2 sequence halves. Core (g, s) computes
query heads {2g, 2g+1} and KV head g for query tokens [s*2048, (s+1)*2048).
Unlike v1 (which recomputed a 1024-token KV halo locally from zero-padded x),
each core projects K/V only for its OWN 2048 tokens; the halo K/V for the
second-half cores is delivered by a pair AllReduce(add): every core contributes
its K/V tiles T2,T3 (tokens [s*2048+1024, (s+1)*2048)) multiplied by a
per-core host mask (1.0 on s=0 cores, 0.0 on s=1), so the pair-sum equals the
s=0 core's tiles = exactly the halo the s=1 core needs. s=0 cores receive
their own tiles back into the halo slots, which their kbias (-1e5 on jt<8)
already masks out — the program stays fully SPMD.

Schedule: KV+Q projections for tiles [2,3,0,1] (halo contribution first so the
collective overlaps the remaining projections), then attention+o_proj for
query blocks [2,3,0,1] (blocks 0,1 need the halo and run last).

DMA: sync queue carries the deadline-ordered input stream (wk/wv, x tiles,
rope tables, consts); gpsimd SW-DGE carries bulky weights (wq/tri/wo), the
collective in/out staging, and all o_part output writes (bf16; host sums).
"""

import sys

if "/opt/trn_rl_repo" not in sys.path:
    sys.path.insert(0, "/opt/trn_rl_repo")

import numpy as np

try:
    import ml_dtypes
    BF16 = ml_dtypes.bfloat16
except ImportError:
    BF16 = None

T, DM, NH, NKV, HD, WIN = 4096, 2048, 8, 4, 256, 1024
EPS, BASE = 1e-6, 10000.0
NG, NS = 4, 2
NQ = 2048
NTB = 4           # 512-token own-K/V tiles per core
NKO = 16          # 2048 / 128 contraction subtiles
SCALE = 1.0 / 16.0
NEG = -1.0e5

_cache = {}


def _host_prep(x, pos, Wq, Wk, Wv, Wo, q_norm_w, k_norm_w):
    x = np.asarray(x, np.float32).reshape(T, DM)
    xT = np.ascontiguousarray(x.T)
    pos_f = np.asarray(pos).astype(np.float64)
    m = np.arange(128)
    invf = BASE ** (-m / 128.0)

    Wq = np.asarray(Wq, np.float32)
    Wk = np.asarray(Wk, np.float32)
    Wv = np.asarray(Wv, np.float32)
    Wo = np.asarray(Wo, np.float32)
    qnw = np.asarray(q_norm_w, np.float32)
    knw = np.asarray(k_norm_w, np.float32)

    r0T = np.zeros((128, 128), np.float32)
    a = np.arange(64)
    r0T[2 * a, 2 * a + 1] = 1.0
    r0T[2 * a + 1, 2 * a] = -1.0
    qw2 = np.ascontiguousarray(np.stack([qnw[:128], qnw[128:]], axis=1))
    kw2 = np.ascontiguousarray(np.stack([knw[:128], knw[128:]], axis=1))

    # masks for 512-wide attention blocks: m=0..3 far edge, m=8..11 diagonal
    jp = np.arange(128)[:, None]
    ip = np.arange(512)[None, :]
    tris = []
    for mm_ in range(4):
        tris.append(jp >= ip + 1 - 128 * mm_)         # far masks F_m
    for mm_ in range(4):
        tris.append(jp <= ip - 128 * mm_)             # diag masks D_{m+8}
    tri = np.concatenate(tris, axis=1).astype(BF16)   # [128, 8*512]

    in_maps = []
    for g in range(NG):
        wkT = Wk[g * HD:(g + 1) * HD, :].T            # [DM, 256]
        wvT = Wv[g * HD:(g + 1) * HD, :].T
        wqT = Wq[2 * g * HD:(2 * g + 2) * HD, :].T    # [DM, 512]
        woT = Wo[:, 2 * g * HD:(2 * g + 2) * HD].T    # [512, DM]
        wkp = np.ascontiguousarray(
            wkT.reshape(NKO, 128, 256).transpose(1, 0, 2)).astype(BF16)
        wvp = np.ascontiguousarray(
            wvT.reshape(NKO, 128, 256).transpose(1, 0, 2)).astype(BF16)
        wqp = np.ascontiguousarray(
            wqT.reshape(NKO, 128, 512).transpose(1, 0, 2)).astype(BF16)
        wop = np.ascontiguousarray(
            woT.reshape(4, 128, DM).transpose(1, 0, 2)).astype(BF16)
        for s in range(NS):
            lo = s * 2048
            xT_c = xT[:, lo:lo + 2048]
            # [8, 128, 8, 512]: tile (t, half) -> ko = half*8..half*8+7
            xp = np.ascontiguousarray(
                xT_c.reshape(NKO, 128, NTB, 512)
                .transpose(2, 0, 1, 3)                 # [t, ko, p, tok]
                .reshape(NTB, 2, 8, 128, 512)
                .transpose(0, 1, 3, 2, 4)              # [t, half, p, k, tok]
                .reshape(NTB * 2, 128, 8, 512)).astype(BF16)

            p = pos_f[lo:lo + 2048]
            ang = p[None, :] * invf[:, None]
            cosk = np.cos(ang).astype(np.float32)      # [128, 2048]
            sink = np.sin(ang).astype(np.float32)
            cosp = np.ascontiguousarray(
                cosk.reshape(128, NTB, 512).transpose(1, 0, 2))
            sinp = np.ascontiguousarray(
                sink.reshape(128, NTB, 512).transpose(1, 0, 2))

            kbias = np.zeros((128, 24), np.float32)
            if s == 0:
                kbias[:, :8] = NEG
            msk = np.full((128, 1), 1.0 if s == 0 else 0.0, np.float32)

            in_maps.append({
                "xp": xp,
                "cosp": cosp,
                "sinp": sinp,
                "wkp": wkp,
                "wvp": wvp,
                "wqp": wqp,
                "wop": wop,
                "r0T": r0T.astype(BF16),
                "qw": qw2,
                "kw": kw2,
                "kbias": kbias,
                "msk": msk,
                "tri": tri,
            })
    return in_maps


def _build_program():
    if "nc" in _cache:
        return _cache["nc"]

    import concourse.bass as bass
    import concourse.mybir as mybir
    import concourse.tile as tile
    from concourse import bacc
    from contextlib import ExitStack

    f32 = mybir.dt.float32
    bf16 = mybir.dt.bfloat16
    AF = mybir.ActivationFunctionType
    OP = mybir.AluOpType

    nc = bacc.Bacc("TRN2", target_bir_lowering=False, debug=False,
                   enable_asserts=False, num_devices=8)

    xp_d = nc.dram_tensor("xp", [NTB * 2, 128, 8, 512], bf16, kind="ExternalInput")
    cosp_d = nc.dram_tensor("cosp", [NTB, 128, 512], f32, kind="ExternalInput")
    sinp_d = nc.dram_tensor("sinp", [NTB, 128, 512], f32, kind="ExternalInput")
    wq_d = nc.dram_tensor("wqp", [128, NKO, 512], bf16, kind="ExternalInput")
    wk_d = nc.dram_tensor("wkp", [128, NKO, 256], bf16, kind="ExternalInput")
    wv_d = nc.dram_tensor("wvp", [128, NKO, 256], bf16, kind="ExternalInput")
    wo_d = nc.dram_tensor("wop", [128, 4, DM], bf16, kind="ExternalInput")
    r0_d = nc.dram_tensor("r0T", [128, 128], bf16, kind="ExternalInput")
    qw_d = nc.dram_tensor("qw", [128, 2], f32, kind="ExternalInput")
    kw_d = nc.dram_tensor("kw", [128, 2], f32, kind="ExternalInput")
    kb_d = nc.dram_tensor("kbias", [128, 24], f32, kind="ExternalInput")
    msk_d = nc.dram_tensor("msk", [128, 1], f32, kind="ExternalInput")
    tri_d = nc.dram_tensor("tri", [128, 8 * 512], bf16, kind="ExternalInput")
    o_d = nc.dram_tensor("o_part", [NQ, DM], bf16, kind="ExternalOutput")

    cc_in = nc.dram_tensor("cc_in", [128, 4096], bf16, kind="Internal")
    cc_out = nc.dram_tensor("cc_out", [128, 4096], bf16, kind="Internal")

    with tile.TileContext(nc) as tc, ExitStack() as ctx:
        cpool = ctx.enter_context(tc.tile_pool(name="consts", bufs=1))
        xpool = ctx.enter_context(tc.tile_pool(name="xt", bufs=4))
        tabpool = ctx.enter_context(tc.tile_pool(name="tab", bufs=3))
        kvpool = ctx.enter_context(tc.tile_pool(name="kv", bufs=1))
        scpool = ctx.enter_context(tc.tile_pool(name="scratch", bufs=3))
        spool = ctx.enter_context(tc.tile_pool(name="small", bufs=2))
        qpool = ctx.enter_context(tc.tile_pool(name="qt", bufs=1))
        ptpool = ctx.enter_context(tc.tile_pool(name="pt", bufs=4))
        ypool = ctx.enter_context(tc.tile_pool(name="yt", bufs=2))
        opool = ctx.enter_context(tc.tile_pool(name="osb", bufs=3))
        pp_small = ctx.enter_context(tc.tile_pool(name="psmall", bufs=2, space="PSUM"))
        proj_ctx = ExitStack()
        pp_proj = proj_ctx.enter_context(tc.tile_pool(name="pproj", bufs=3, space="PSUM"))

        KVORD = [2, 3, 0, 1]      # projection tile order (halo contribution first)
        AORD = [2, 3, 0, 1]       # attention block order (halo consumers last)

        # ---- sync queue: deadline-ordered latency-critical stream ----
        wk_sb = cpool.tile([128, NKO, 256], bf16, tag="wk")
        nc.sync.dma_start(wk_sb[:, 0:8, :], wk_d.ap()[:, 0:8, :])
        xt_tiles = [None] * (NTB * 2)

        def load_x(t):
            for half in range(2):
                i = t * 2 + half
                xt = xpool.tile([128, 8, 512], bf16, tag="xt", name=f"xt_p{i}")
                nc.sync.dma_start(xt[:], xp_d.ap()[i])
                xt_tiles[i] = xt

        cos_tiles = [None] * NTB
        sin_tiles = [None] * NTB

        def load_tab(t):
            ct = tabpool.tile([128, 512], f32, tag="cos", name=f"cos{t}")
            nc.sync.dma_start(ct[:], cosp_d.ap()[t])
            st = tabpool.tile([128, 512], f32, tag="sin", name=f"sin{t}")
            nc.sync.dma_start(st[:], sinp_d.ap()[t])
            cos_tiles[t] = ct
            sin_tiles[t] = st

        load_x(KVORD[0])
        nc.sync.dma_start(wk_sb[:, 8:16, :], wk_d.ap()[:, 8:16, :])
        load_tab(KVORD[0])
        wq_sb = cpool.tile([128, NKO, 512], bf16, tag="wq")
        nc.sync.dma_start(wq_sb[:, 0:8, :], wq_d.ap()[:, 0:8, :])
        nc.sync.dma_start(wq_sb[:, 8:16, :], wq_d.ap()[:, 8:16, :])
        r0_sb = cpool.tile([128, 128], bf16, tag="r0")
        nc.sync.dma_start(r0_sb[:], r0_d.ap())
        qw_sb = cpool.tile([128, 2], f32, tag="qwt")
        nc.sync.dma_start(qw_sb[:], qw_d.ap())
        kw_sb = cpool.tile([128, 2], f32, tag="kwt")
        nc.sync.dma_start(kw_sb[:], kw_d.ap())
        kb_sb = cpool.tile([128, 24], f32, tag="kb")
        nc.sync.dma_start(kb_sb[:], kb_d.ap())
        msk_sb = cpool.tile([128, 1], f32, tag="msk")
        nc.sync.dma_start(msk_sb[:], msk_d.ap())
        wv_sb = cpool.tile([128, NKO, 256], bf16, tag="wv")
        nc.sync.dma_start(wv_sb[:, 0:8, :], wv_d.ap()[:, 0:8, :])
        nc.sync.dma_start(wv_sb[:, 8:16, :], wv_d.ap()[:, 8:16, :])
        load_x(KVORD[1])
        load_tab(KVORD[1])
        load_x(KVORD[2])
        load_tab(KVORD[2])
        load_x(KVORD[3])
        load_tab(KVORD[3])
        # latency-tolerant weights stream after the proj-phase inputs
        tri_sb = cpool.tile([128, 8 * 512], bf16, tag="tri")
        nc.sync.dma_start(tri_sb[:], tri_d.ap())
        wo_sb = cpool.tile([128, 4, DM], bf16, tag="wo")
        nc.sync.dma_start(wo_sb[:, 0:2, :], wo_d.ap()[:, 0:2, :])
        nc.sync.dma_start(wo_sb[:, 2:4, :], wo_d.ap()[:, 2:4, :])

        ones_sb = cpool.tile([128, 128], bf16, tag="ones")
        nc.vector.memset(ones_sb[:], 1.0)
        eps_sb = cpool.tile([128, 1], f32, tag="eps")
        nc.vector.memset(eps_sb[:], EPS)
        zero_sb = cpool.tile([128, 1], f32, tag="zero")
        nc.vector.memset(zero_sb[:], 0.0)

        # K/V slots: 0,1 = halo (filled by the collective), 2..5 = own tiles
        kt_tiles = []
        vt_tiles = []
        for sl in range(6):
            ktile = kvpool.tile([128, 2, 512], bf16, tag=f"kt{sl}", name=f"kt{sl}")
            kt_tiles.append(ktile)
            vtile = kvpool.tile([128, 4, 256], bf16, tag=f"vt{sl}", name=f"vt{sl}")
            vt_tiles.append(vtile)
        qt_tiles = [None] * NTB

        def norm_rope(src_ps, w_sb, cos_t, sin_t, dst, dsti):
            z2 = scpool.tile([128, 2, 512], bf16, tag="z2")
            for u in range(2):
                nc.scalar.activation(z2[:, u, :], src_ps[u][:], AF.Square,
                                     bias=zero_sb[:])
            ssq = pp_small.tile([128, 512], f32, tag="psm")
            for u in range(2):
                nc.tensor.matmul(ssq[:], ones_sb[:], z2[:, u, :],
                                 start=(u == 0), stop=(u == 1))
            sq = spool.tile([128, 512], f32, tag="sq")
            nc.scalar.activation(sq[:], ssq[:], AF.Sqrt, bias=eps_sb[:], scale=1.0 / HD)
            rs = spool.tile([128, 512], f32, tag="rs")
            nc.vector.reciprocal_approx_fast(rs[:], sq[:])
            znw = scpool.tile([128, 2, 512], bf16, tag="znw")
            t1 = scpool.tile([128, 2, 512], f32, tag="t1")
            for u in range(2):
                nc.vector.scalar_tensor_tensor(
                    znw[:, u, :], src_ps[u][:], w_sb[:, u:u + 1], rs[:],
                    OP.mult, OP.mult)
                rot = pp_small.tile([128, 512], f32, tag="psm")
                nc.tensor.matmul(rot[:], r0_sb[:], znw[:, u, :], start=True, stop=True)
                nc.vector.tensor_tensor(t1[:, u, :], znw[:, u, :], cos_t, OP.mult)
                tmp = spool.tile([128, 512], f32, tag="tmp")
                nc.vector.tensor_tensor(tmp[:], rot[:], sin_t, OP.mult)
                nc.vector.tensor_tensor(dst[:, dsti + u, :], t1[:, u, :], tmp[:], OP.add)

        # ---- projection phase: tiles in KVORD; Q right after its K/V ----
        for t in KVORD:
            xth = xt_tiles[t * 2:t * 2 + 2]
            cos_t = cos_tiles[t]
            sin_t = sin_tiles[t]
            sl = t + 2            # own tile t lands in halo-space slot t+2

            # K projection (transposed, N=512), ko-inner
            k0_ps = pp_proj.tile([128, 512], f32, tag="pj")
            k1_ps = pp_proj.tile([128, 512], f32, tag="pj")
            k_ps = [k0_ps, k1_ps]
            for ko in range(NKO):
                for dsub in range(2):
                    nc.tensor.matmul(k_ps[dsub][:],
                                     wk_sb[:, ko, dsub * 128:(dsub + 1) * 128],
                                     xth[ko // 8][:, ko % 8, :],
                                     start=(ko == 0), stop=(ko == NKO - 1),
                                     skip_group_check=True)
            norm_rope(k_ps, kw_sb, cos_t[:], sin_t[:], kt_tiles[sl], 0)

            # Q projections (2 heads, N=512)
            qt_sb = qpool.tile([128, 4, 512], bf16, tag=f"q{t}", name=f"qt{t}")
            qt_tiles[t] = qt_sb
            for h in range(2):
                q0_ps = pp_proj.tile([128, 512], f32, tag="pj")
                q1_ps = pp_proj.tile([128, 512], f32, tag="pj")
                q_ps = [q0_ps, q1_ps]
                for u in range(2):
                    dsub = 2 * h + u
                    for ko in range(NKO):
                        nc.tensor.matmul(q_ps[u][:],
                                         wq_sb[:, ko, dsub * 128:(dsub + 1) * 128],
                                         xth[ko // 8][:, ko % 8, :],
                                         start=(ko == 0), stop=(ko == NKO - 1))
                norm_rope(q_ps, qw_sb, cos_t[:], sin_t[:], qt_sb, 2 * h)

            # V projection (natural layout); released fast by the copies, so
            # the next tile's first PSUM allocations never stall
            vt = vt_tiles[sl]
            for vh in range(2):
                v_ps = pp_proj.tile([128, 2, 256], f32, tag="pj")
                for ms in range(2):
                    msub = vh * 2 + ms
                    for ko in range(NKO):
                        nc.tensor.matmul(v_ps[:, ms, :],
                                         xth[ko // 8][:, ko % 8, msub * 128:(msub + 1) * 128],
                                         wv_sb[:, ko, :],
                                         start=(ko == 0), stop=(ko == NKO - 1))
                for ms in range(2):
                    nc.vector.tensor_copy(vt[:, vh * 2 + ms, :], v_ps[:, ms, :])

            if t == KVORD[1]:
                # halo contribution: tiles T2,T3 (slots 4,5) masked by msk
                contrib = cpool.tile([128, 4, 1024], bf16, tag="contrib")
                nc.vector.tensor_scalar(
                    contrib[:, 0, :], kt_tiles[4][:].rearrange("p a b -> p (a b)"),
                    msk_sb[:], None, OP.mult)
                nc.vector.tensor_scalar(
                    contrib[:, 1, :], kt_tiles[5][:].rearrange("p a b -> p (a b)"),
                    msk_sb[:], None, OP.mult)
                nc.vector.tensor_scalar(
                    contrib[:, 2, :], vt_tiles[4][:].rearrange("p a b -> p (a b)"),
                    msk_sb[:], None, OP.mult)
                nc.vector.tensor_scalar(
                    contrib[:, 3, :], vt_tiles[5][:].rearrange("p a b -> p (a b)"),
                    msk_sb[:], None, OP.mult)
                nc.sync.dma_start(cc_in.ap(), contrib[:])
                nc.gpsimd.collective_compute(
                    "AllReduce", OP.add,
                    replica_groups=[[0, 1], [2, 3], [4, 5], [6, 7]],
                    ins=[cc_in.ap()],
                    outs=[cc_out.ap()],
                )
                nc.sync.dma_start(kt_tiles[0][:], cc_out.ap()[:, 0:1024])
                nc.sync.dma_start(kt_tiles[1][:], cc_out.ap()[:, 1024:2048])
                nc.sync.dma_start(vt_tiles[0][:], cc_out.ap()[:, 2048:3072])
                nc.sync.dma_start(vt_tiles[1][:], cc_out.ap()[:, 3072:4096])

        proj_ctx.close()
        pp_acc = ctx.enter_context(tc.tile_pool(name="pacc", bufs=6, space="PSUM"))

        # ---- attention per block; o-projection runs one block behind so
        # ready o_proj matmuls fill the TE bubbles around each block's
        # softmax-denominator/yt DVE chain ----
        yt_blocks = {}

        def o_proj(a):
            yt_sb = yt_blocks.pop(a)
            for msub in range(4):
                for dmh in range(2):
                    o_sb = opool.tile([128, 1024], bf16, tag="o")
                    for dq in range(2):
                        c0 = (dmh * 2 + dq) * 512
                        o_ps = pp_small.tile([128, 512], f32, tag="psm")
                        for hd in range(4):
                            nc.tensor.matmul(o_ps[:],
                                             yt_sb[:, hd, msub * 128:(msub + 1) * 128],
                                             wo_sb[:, hd, c0:c0 + 512],
                                             start=(hd == 0), stop=(hd == 3))
                        nc.scalar.copy(o_sb[:, dq * 512:(dq + 1) * 512], o_ps[:])
                    r0_ = a * 512 + msub * 128
                    nc.scalar.dma_start(o_d.ap()[r0_:r0_ + 128, dmh * 1024:(dmh + 1) * 1024],
                                        o_sb[:])

        for ai, a in enumerate(AORD):
            qt_sb = qt_tiles[a]
            yt_sb = ypool.tile([128, 4, 512], bf16, tag="y")
            yt_blocks[a] = yt_sb
            for h in range(2):
                dn_ps = pp_acc.tile([128, 512], f32, tag="pac")
                y0_ps = pp_acc.tile([128, 512], f32, tag="pac")
                y1_ps = pp_acc.tile([128, 512], f32, tag="pac")
                y_ps = [y0_ps, y1_ps]
                for mi, mrel in enumerate([3, 0, 1, 2] + list(range(4, 12))):
                    jt = 4 * a + mrel
                    ct, jh = jt // 4, jt % 4
                    ktc = kt_tiles[ct]
                    vtc = vt_tiles[ct]
                    if mrel <= 2:
                        ia, ib = 0, 128 * (mrel + 1)
                    elif mrel >= 9:
                        ia, ib = 128 * (mrel - 8), 512
                    else:
                        ia, ib = 0, 512
                    pt = ptpool.tile([128, 512], bf16, tag="p")
                    st = pp_small.tile([128, 512], f32, tag="psm")
                    for u in range(2):
                        nc.tensor.matmul(st[:, ia:ib],
                                         ktc[:, u, jh * 128:(jh + 1) * 128],
                                         qt_sb[:, 2 * h + u, ia:ib],
                                         start=(u == 0), stop=(u == 1))
                    nc.scalar.activation(pt[:, ia:ib], st[:, ia:ib], AF.Exp,
                                         bias=kb_sb[:, jt:jt + 1], scale=SCALE)
                    if mrel < 4:
                        nc.vector.tensor_tensor(
                            pt[:, ia:ib], pt[:, ia:ib],
                            tri_sb[:, mrel * 512 + ia:mrel * 512 + ib], OP.mult)
                    elif mrel >= 8:
                        nc.vector.tensor_tensor(
                            pt[:, ia:ib], pt[:, ia:ib],
                            tri_sb[:, (mrel - 4) * 512 + ia:(mrel - 4) * 512 + ib],
                            OP.mult)
                    first, last = (mi == 0), (mrel == 11)
                    nc.tensor.matmul(dn_ps[:, ia:ib], ones_sb[:], pt[:, ia:ib],
                                     start=first, stop=last, skip_group_check=True)
                    for dh in range(2):
                        nc.tensor.matmul(y_ps[dh][:, ia:ib],
                                         vtc[:, jh, dh * 128:(dh + 1) * 128],
                                         pt[:, ia:ib], start=first, stop=last,
                                         skip_group_check=True)
                rc = spool.tile([128, 512], f32, tag="rc")
                nc.vector.reciprocal_approx_fast(rc[:], dn_ps[:])
                for dh in range(2):
                    nc.vector.tensor_tensor(yt_sb[:, 2 * h + dh, :],
                                            y_ps[dh][:], rc[:], OP.mult)

            if ai > 0:
                o_proj(AORD[ai - 1])
        o_proj(AORD[-1])

    nc.compile()
    _cache["nc"] = nc
    return nc


def _run(inputs, trace=False):
    from concourse.bass_utils import run_bass_kernel_spmd

    nc = _build_program()
    in_maps = _host_prep(**inputs)
    res = run_bass_kernel_spmd(nc, in_maps, core_ids=list(range(8)), trace=trace)
    full = np.zeros((T, DM), np.float32)
    for g in range(NG):
        for s in range(NS):
            full[s * 2048:(s + 1) * 2048] += res.results[g * 2 + s]["o_part"].astype(np.float32)
    return full.reshape(1, T, DM), res


def kernel(**inputs):
    return _run(inputs, trace=False)[0]
